# revision 38
# baseline (speedup 1.0000x reference)
"""Trainium2 Bass kernel for causal MultiHeadAttention (B=2, S=2048, E=1024, H=16).

Sharding: 8 cores = 2 (batch) x 4 (head groups of 4, Megatron-style).
Each core computes, for its batch b and head group g:
  - Q/K projections into transposed layout qhT/khT [256, S]  (256 = 4 heads x 64)
  - V projection into natural layout vh [S, 256] with a ones-column per head
  - causal attention with scores kept transposed [k, q]; softmax denominators
    come out of the PV matmul via the ones-column; no max-subtraction needed
    (|scores/sqrt(D)| <~ 6 so exp is well within fp32 range; masked entries are
    zeroed AFTER exp, which matches the reference's -1e9 masking exactly)
  - partial output projection attn_concat @ Wo[rows of g]  -> [S, E]
Host sums the 4 partials per batch and adds bo.

All matmul operands are float16 (full PE rate, fp32 PSUM accumulation).
Schedule notes:
  - Prologue input DMAs are split across the two HWDGE queues (SP and ACT;
    each dispatch costs ~700ns of serial queue time, and ACT is idle during
    the prologue).  Steady-state loads stay on SP; the final output DMAs
    alternate queues again.
  - Causal masks are built on-device (memset + affine_select), ones tiles
    via memset: no mask/ones DMA.
  - x and weights arrive PRE-TILED from the host ([qc, p, kt, nq] / [p, kt, m])
    so every input DMA is contiguous per partition: 8KB descriptors instead
    of 1KB (x) / 512B (weights) strided rows — measured ~2us faster.
  - x chunks are prefetched TWO rounds ahead (xpool bufs=9) so projection
    matmuls never wait on HBM arrivals mid-kernel.
  - No PE warmup at t~0: warmup matmuls measured ~1.5us SLOWER (they feed
    the HAM power governor, which then clamps rounds 0-1 to half rate).
  - Prologue projection matmuls are emitted kt-major (m inner) so a freshly
    arriving x-tile is consumed by both m-blocks back to back, halving the
    HBM rate the prologue needs.
  - Q rounds are 4 x 512 (splitting rounds was measured ~6-10us slower:
    every extra round re-loads all prior k-block stationaries, ~400ns fixed
    PE cost per (head-pair, k-block) unit).
  - Output tiles are [128, 1024] per row-block: two casts + ONE output DMA
    (halves the ~600ns/dispatch load on the SP queue).
  - Final-round tail: wo kt0 matmuls are emitted BEFORE the last head-pair's
    normalization thunks (PE-queue order is execution order, so emitting
    them later serializes them behind the norm), the 1/denominator
    broadcast is a rank-1 PE matmul into a borrowed scores-psum bank
    instead of a gpsimd partition_broadcast, and the final psum->sbuf casts
    alternate between DVE and ACT.
"""

import numpy as np

B, S, E, H = 2, 2048, 1024, 16
D = E // H            # 64 head dim
HL = 4                # heads per core
CW = HL * D           # 256 local channels
P = 128
KT = E // P           # 8 contraction tiles for the input projections
D1 = D + 1            # head slot in vh (+ ones column)
NQMAX = 512

ROUNDS = [(0, 512), (512, 512), (1024, 512), (1536, 512)]
# round r -> earlier rounds whose output projection runs as its filler
WO_SCHED = {2: [0], 3: [1, 2]}

_CACHE = {}


def _pin_act_table(mybir, bacc):
    """Force all activations onto one LUT set containing exp+ln+identity, so
    the ACT engine never reloads tables mid-kernel (1.3us per reload)."""
    from concourse.hw_specs import get_activation_tables

    need = {
        mybir.ActivationFunctionType.Exp,
        mybir.ActivationFunctionType.Ln,
        mybir.ActivationFunctionType.Identity,
    }
    orig = get_activation_tables("gen3")
    target = next(n for n, fs in orig.items() if need <= fs)
    pinned = {n: (fs if n == target else set()) for n, fs in orig.items()}
    bacc.get_activation_tables = lambda arch: pinned


def _build(num_devices=8):
    import concourse.mybir as mybir
    import concourse.tile as tile
    from concourse import bacc

    _pin_act_table(mybir, bacc)

    f32 = mybir.dt.float32
    h16 = mybir.dt.float16
    Identity = mybir.ActivationFunctionType.Identity
    Ln = mybir.ActivationFunctionType.Ln
    Exp = mybir.ActivationFunctionType.Exp

    nc = bacc.Bacc(
        "TRN2", target_bir_lowering=False, debug=False, num_devices=num_devices
    )

    def din(name, shape, dt=f32):
        return nc.dram_tensor(name, list(shape), dt, kind="ExternalInput").ap()

    # x and weights arrive pre-tiled from the host so every DMA is
    # contiguous per partition (8KB descriptors instead of 1KB/512B rows)
    QC = S // NQMAX
    xqt = din("xqt", (QC, P, KT, NQMAX), h16)
    xkt = din("xkt", (QC, P, KT, NQMAX), h16)
    xvt = din("xvt", (QC, P, KT, NQMAX), h16)
    wq = din("wq", (P, KT, CW), h16)
    wk = din("wk", (P, KT, CW), h16)
    wv = din("wv", (P, KT, CW), h16)
    wo = din("wo", (P, CW // P, E), h16)
    bq = din("bq", (CW,))
    bk = din("bk", (CW,))
    bv = din("bv", (CW,), h16)
    out = nc.dram_tensor("out", [S, E], h16, kind="ExternalOutput").ap()

    SB = S // P  # 16 k-blocks total

    with tile.TileContext(nc) as tc:
        with (
            tc.tile_pool(name="singles", bufs=1) as singles,
            tc.tile_pool(name="xpool", bufs=9) as xpool,
            tc.tile_pool(name="exp", bufs=10) as exp_pool,
            tc.tile_pool(name="outp", bufs=4) as out_pool,
            tc.tile_pool(name="small", bufs=4) as small_pool,
            tc.tile_pool(name="stage", bufs=6) as stage_pool,
            tc.tile_pool(name="proj_ps", bufs=2, space="PSUM") as proj_ps,
            tc.tile_pool(name="scores_ps", bufs=2, space="PSUM") as scores_ps,
            tc.tile_pool(name="attn_ps", bufs=2, space="PSUM") as attn_ps,
        ):
            dma = nc.sync.dma_start       # steady-state loads
            dma2 = nc.scalar.dma_start    # second HWDGE queue: prologue only
            dma_out = nc.sync.dma_start

            # --- persistent SBUF tensors -------------------------------------
            wq_sb = singles.tile([P, KT, CW], h16, tag="wq")
            wk_sb = singles.tile([P, KT, CW], h16, tag="wk")
            wv_sb = singles.tile([P, KT, CW], h16, tag="wv")
            wo_sb = singles.tile([P, CW // P, E], h16, tag="wo")
            masks_sb = singles.tile([P, 4, NQMAX], h16, tag="masks")
            bq_sb = singles.tile([P, 2], f32, tag="bq")
            bk_sb = singles.tile([P, 2], f32, tag="bk")
            bv_row = singles.tile([1, CW], h16, tag="bv")
            ones_col = singles.tile([1, P], h16, tag="ones")

            qhT = [singles.tile([P, S], h16, name=f"qhT{m}", tag=f"qhT{m}") for m in range(2)]
            khT = [singles.tile([P, S], h16, name=f"khT{m}", tag=f"khT{m}") for m in range(2)]
            atT = [singles.tile([P, S], h16, name=f"atT{m}", tag=f"atT{m}") for m in range(2)]
            vh = singles.tile([P, SB, HL, D1], h16, tag="vh")

            def t_consts():
                # no DMA dependencies: runs at t=0 on otherwise-idle engines
                nc.vector.memset(ones_col, 1.0)
                nc.vector.memset(vh[:, :, :, D:D1], 1.0)
                # causal masks on-device: masks_sb[p, jj, q] = (q >= p + 128*jj)
                nc.vector.memset(masks_sb, 1.0)
                nc.gpsimd.affine_select(
                    out=masks_sb,
                    in_=masks_sb,
                    pattern=[[-P, 4], [1, NQMAX]],
                    channel_multiplier=-1,
                    base=0,
                    compare_op=mybir.AluOpType.is_ge,
                    fill=0.0,
                )

            def t_wk():
                dma(out=wk_sb[:, :1, :], in_=wk[:, :1, :])
                dma(out=wk_sb[:, 1 : KT // 2, :], in_=wk[:, 1 : KT // 2, :])
                dma(out=wk_sb[:, KT // 2 :, :], in_=wk[:, KT // 2 :, :])
                dma(out=bk_sb, in_=bk.rearrange("(m p) -> p m", p=P))

            def t_wv():
                dma2(out=wv_sb, in_=wv)
                dma2(out=bv_row, in_=bv.unsqueeze(0))

            def t_wq():
                dma2(out=wq_sb, in_=wq)
                dma2(out=bq_sb, in_=bq.rearrange("(m p) -> p m", p=P))

            def t_wo():
                dma(out=wo_sb, in_=wo)

            # --- stage helpers (thunk-list builders) -------------------------
            def load_x_thunk(src, qa, nq, holder, key, eng=None):
                def t():
                    d = eng or dma
                    tl = xpool.tile([P, KT, NQMAX], h16, name="xchunk", tag="xchunk")
                    rsrc = src[qa // NQMAX]
                    h = KT // 2
                    if eng is not None:
                        d(out=tl[:, :1, :nq], in_=rsrc[:, :1, :nq])
                        d(out=tl[:, 1:h, :nq], in_=rsrc[:, 1:h, :nq])
                    else:
                        d(out=tl[:, :h, :nq], in_=rsrc[:, :h, :nq])
                    d(out=tl[:, h:, :nq], in_=rsrc[:, h:, :nq])
                    holder[key] = tl
                return [t]

            def proj_qk_thunks(qa, nq, holder, key, w_sb, b_sb, dstT, kt_major=False):
                """kt_major: emit (kt0,m0),(kt0,m1),(kt1,m0)... so each x-tile
                is consumed by both m-blocks back-to-back (halves the HBM rate
                the prologue needs)."""
                thunks = []
                pss = {}
                def mk_mm(m, kt):
                    def t():
                        if kt == 0:
                            pss[m] = proj_ps.tile([P, NQMAX], f32, name="proj", tag="proj")
                        nc.tensor.matmul(
                            pss[m][:, :nq],
                            w_sb[:, kt, m * P : (m + 1) * P],
                            holder[key][:, kt, :nq],
                            start=(kt == 0),
                            stop=(kt == KT - 1),
                        )
                    return t
                def mk_copy(m):
                    def t():
                        nc.vector.tensor_scalar_add(
                            out=dstT[m][:, qa : qa + nq],
                            in0=pss[m][:, :nq],
                            scalar1=b_sb[:, m : m + 1],
                        )
                    return t
                if kt_major:
                    for kt in range(KT):
                        for m in range(2):
                            thunks.append(mk_mm(m, kt))
                    thunks.append(mk_copy(0))
                    thunks.append(mk_copy(1))
                else:
                    for m in range(2):
                        for kt in range(KT):
                            thunks.append(mk_mm(m, kt))
                        thunks.append(mk_copy(m))
                return thunks

            def proj_v_thunks(qa, nq, holder, key):
                thunks = []
                pss = {}
                for mb in range(nq // P):
                    j = qa // P + mb
                    def mk_mm(mb, kt):
                        def t():
                            if kt == 0:
                                pss[mb] = proj_ps.tile([P, NQMAX], f32, name="proj", tag="proj")
                            nc.tensor.matmul(
                                pss[mb][:, :CW],
                                holder[key][:, kt, mb * P : (mb + 1) * P],
                                wv_sb[:, kt, :],
                                start=(kt == 0),
                                stop=False,
                            )
                        return t
                    for kt in range(KT):
                        thunks.append(mk_mm(mb, kt))
                    def mk_tail(mb, j):
                        def t():
                            nc.tensor.matmul(
                                pss[mb][:, :CW],
                                ones_col,
                                bv_row,
                                start=False,
                                stop=True,
                            )
                            nc.vector.tensor_copy(
                                out=vh[:, j, :, 0:D],
                                in_=pss[mb][:, :CW].rearrange("p (h d) -> p h d", h=HL),
                            )
                        return t
                    thunks.append(mk_tail(mb, j))
                return thunks

            def attn_jloop_thunks(qa, nq, hp, ats):
                thunks = []
                cbase = qa // P
                nblk = (qa + nq) // P
                scale = float(1.0 / np.sqrt(D))
                def mk_j(j):
                    def t():
                        if j == 0:
                            ats[0] = attn_ps.tile([D1, NQMAX], f32, name="attn", tag="attn")
                            ats[1] = attn_ps.tile([D1, NQMAX], f32, name="attn", tag="attn")
                        jj = j - cbase
                        q0 = jj * P if jj > 0 else 0
                        sc2 = scores_ps.tile([P, 2, NQMAX], f32, name="sc2", tag="sc2")
                        for hh in range(2):
                            po = hh * D
                            nc.tensor.matmul(
                                sc2[:, hh, q0:nq],
                                khT[hp][po : po + D, j * P : (j + 1) * P],
                                qhT[hp][po : po + D, qa + q0 : qa + nq],
                                start=True,
                                stop=True,
                            )
                        ex2 = exp_pool.tile([P, 2, NQMAX], h16, name="ex2", tag="ex2")
                        nc.scalar.activation(
                            out=ex2[:, :, q0:nq], in_=sc2[:, :, q0:nq], func=Exp,
                            scale=scale,
                        )
                        if jj >= 0:
                            for hh in range(2):
                                exh = ex2[:, hh, q0:nq]
                                nc.vector.tensor_mul(exh, exh, masks_sb[:, jj, q0:nq])
                        for hh in range(2):
                            nc.tensor.matmul(
                                ats[hh][:, q0:nq],
                                vh[:, j, 2 * hp + hh, :],
                                ex2[:, hh, q0:nq],
                                start=(j == 0),
                                stop=(j == nblk - 1),
                            )
                    return t
                for j in range(nblk):
                    thunks.append(mk_j(j))
                return thunks

            def attn_norm_thunks(qa, nq, hp, ats):
                """Standard (mid-kernel) normalization: ln -> copy-out ->
                exp(-x) -> gpsimd broadcast -> multiply into atT."""
                thunks = []
                atu = {}
                lns = {}
                def mk_stage(hh):
                    def t():
                        ls = small_pool.tile([1, NQMAX], f32, name="ls", tag="ls")
                        nc.scalar.activation(
                            out=ls[:, :nq], in_=ats[hh][D : D + 1, :nq], func=Ln,
                            scale=1.0,
                        )
                        lns[hh] = ls
                        atu[hh] = stage_pool.tile([D, NQMAX], h16, name="atu", tag="atu")
                        nc.vector.tensor_copy(atu[hh][:, :nq], ats[hh][0:D, :nq])
                    return t
                def mk_norm(hh):
                    def t():
                        po = hh * D
                        rs = small_pool.tile([1, NQMAX], f32, name="rs", tag="rs")
                        nc.scalar.activation(
                            out=rs[:, :nq], in_=lns[hh][:, :nq], func=Exp, scale=-1.0
                        )
                        rb = small_pool.tile([D, NQMAX], f32, name="rb", tag="rb")
                        nc.gpsimd.partition_broadcast(rb[:, :nq], rs[:, :nq])
                        nc.vector.tensor_mul(
                            atT[hp][po : po + D, qa : qa + nq],
                            atu[hh][:, :nq],
                            rb[:, :nq],
                        )
                    return t
                thunks.append(mk_stage(0))
                thunks.append(mk_stage(1))
                thunks.append(mk_norm(0))
                thunks.append(mk_norm(1))
                return thunks

            def attn_norm_fast_thunks(qa, nq, hp, ats, rb_ps_holder):
                """Final-round normalization for the last head pair: ACT chain
                ordered ln0,exp0,ln1,exp1; 1/denominator broadcast via a
                rank-1 PE matmul into a borrowed scores bank; staging copies
                run on DVE in parallel with the ACT chain."""
                thunks = []
                lns = {}
                rss = {}
                atu = {}
                def mk_ln(hh):
                    def t():
                        ls = small_pool.tile([1, NQMAX], f32, name="ls", tag="ls")
                        nc.scalar.activation(
                            out=ls[:, :nq], in_=ats[hh][D : D + 1, :nq], func=Ln,
                            scale=1.0,
                        )
                        lns[hh] = ls
                        atu[hh] = stage_pool.tile([D, NQMAX], h16, name="atu", tag="atu")
                        nc.vector.tensor_copy(atu[hh][:, :nq], ats[hh][0:D, :nq])
                    return t
                def mk_exp(hh):
                    def t():
                        rs = small_pool.tile([1, NQMAX], h16, name="rs16", tag="rs16")
                        nc.scalar.activation(
                            out=rs[:, :nq], in_=lns[hh][:, :nq], func=Exp, scale=-1.0
                        )
                        rss[hh] = rs
                    return t
                def mk_rb(hh):
                    def t():
                        if hh == 0:
                            rb_ps_holder[0] = scores_ps.tile(
                                [P, 2, NQMAX], f32, name="sc2", tag="sc2"
                            )
                        nc.tensor.matmul(
                            rb_ps_holder[0][0:D, hh, :nq],
                            ones_col[0:1, 0:D],
                            rss[hh][:, :nq],
                            start=True,
                            stop=True,
                        )
                    return t
                def mk_mul(hh):
                    def t():
                        po = hh * D
                        nc.vector.tensor_mul(
                            atT[hp][po : po + D, qa : qa + nq],
                            atu[hh][:, :nq],
                            rb_ps_holder[0][0:D, hh, :nq],
                        )
                    return t
                # ACT chain first (immediately runnable), PE/DVE parts later
                # so PE filler emitted in between does not serialize the chain
                act_part = [mk_ln(0), mk_exp(0), mk_ln(1), mk_exp(1)]
                mul_part = [mk_rb(0), mk_mul(0), mk_rb(1), mk_mul(1)]
                return act_part, mul_part

            def wo_thunks(qa, nq):
                """Output projection per 128-row block ms: two [P, NQMAX]
                psum halves, two casts, ONE [P, E] output DMA."""
                thunks = []
                for mb in range(nq // P):
                    ms = qa // P + mb
                    def mk(ms):
                        def t():
                            ot = out_pool.tile([P, E], h16, name="ot", tag="ot")
                            for n in range(2):
                                ps = proj_ps.tile([P, NQMAX], f32, name="proj", tag="proj")
                                for kt in range(CW // P):
                                    nc.tensor.matmul(
                                        ps,
                                        atT[kt][:, ms * P : (ms + 1) * P],
                                        wo_sb[:, kt, n * NQMAX : (n + 1) * NQMAX],
                                        start=(kt == 0),
                                        stop=(kt == CW // P - 1),
                                    )
                                nc.vector.tensor_copy(
                                    ot[:, n * NQMAX : (n + 1) * NQMAX], ps
                                )
                            dma_out(out=out[ms * P : (ms + 1) * P, :], in_=ot)
                        return t
                    thunks.append(mk(ms))
                return thunks

            def wo_tail_thunks(qa, nq):
                """Final-round wo as (pre, post): `pre` = kt0 matmuls of the
                first wave (only need atT[0]); `post` = the rest.  Casts
                alternate DVE/ACT; one DMA per 128-row block."""
                mss = [qa // P + mb for mb in range(nq // P)]
                pre, post = [], []
                pss = {}
                ots = {}
                sc_shared = {}
                def mk_kt0(u, ms, n):
                    def t():
                        i = 2 * u + n
                        if i % 4 < 2:
                            pss[i] = proj_ps.tile([P, NQMAX], f32, name="proj", tag="proj")
                        else:
                            if i % 4 == 2:
                                sc_shared[i // 4] = scores_ps.tile(
                                    [P, 2, NQMAX], f32, name="sc2", tag="sc2"
                                )
                            pss[i] = sc_shared[i // 4][:, i % 2, :]
                        nc.tensor.matmul(
                            pss[i],
                            atT[0][:, ms * P : (ms + 1) * P],
                            wo_sb[:, 0, n * NQMAX : (n + 1) * NQMAX],
                            start=True,
                            stop=False,
                        )
                    return t
                def mk_kt1(u, ms, n):
                    def t():
                        i = 2 * u + n
                        if n == 0:
                            ots[u] = out_pool.tile([P, E], h16, name="ot", tag="ot")
                        nc.tensor.matmul(
                            pss[i],
                            atT[1][:, ms * P : (ms + 1) * P],
                            wo_sb[:, 1, n * NQMAX : (n + 1) * NQMAX],
                            start=False,
                            stop=True,
                        )
                        dst = ots[u][:, n * NQMAX : (n + 1) * NQMAX]
                        if n == 0:
                            nc.vector.tensor_copy(dst, pss[i])
                        else:
                            nc.scalar.activation(
                                out=dst, in_=pss[i], func=Identity, scale=1.0
                            )
                            # alternate the final output dispatches across the
                            # two HWDGE queues (~600ns of queue time each)
                            eng = dma_out if u % 2 == 0 else dma2
                            eng(out=out[ms * P : (ms + 1) * P, :], in_=ots[u])
                    return t
                # wave A: units 0,1 (4 psum banks); wave B: units 2,3
                for u, ms in enumerate(mss[:2]):
                    for n in range(2):
                        pre.append(mk_kt0(u, ms, n))
                for u, ms in enumerate(mss[:2]):
                    for n in range(2):
                        post.append(mk_kt1(u, ms, n))
                for u, ms in enumerate(mss[2:], start=2):
                    for n in range(2):
                        post.append(mk_kt0(u, ms, n))
                for u, ms in enumerate(mss[2:], start=2):
                    for n in range(2):
                        post.append(mk_kt1(u, ms, n))
                return pre, post

            def emit_interleaved(primary, filler):
                fi = 0
                n = max(len(primary), 1)
                f = len(filler)
                for i, t in enumerate(primary):
                    t()
                    while fi * n < f * (i + 1):
                        filler[fi]()
                        fi += 1
                for t in filler[fi:]:
                    t()

            # --- main schedule ----------------------------------------------
            holder = {}
            qa0, nq0 = ROUNDS[0]
            prologue = (
                [t_consts, t_wk]
                + load_x_thunk(xkt, qa0, nq0, holder, ("xk", 0), eng=dma2)
                + proj_qk_thunks(qa0, nq0, holder, ("xk", 0), wk_sb, bk_sb, khT, kt_major=True)
                + [t_wv]
                + load_x_thunk(xvt, qa0, nq0, holder, ("xv", 0), eng=dma)
                + proj_v_thunks(qa0, nq0, holder, ("xv", 0))
                + [t_wq]
                + load_x_thunk(xqt, qa0, nq0, holder, ("xq", 0), eng=dma2)
                + proj_qk_thunks(qa0, nq0, holder, ("xq", 0), wq_sb, bq_sb, qhT, kt_major=True)
            )
            for t in prologue:
                t()
            kv_deferred = {}
            NR = len(ROUNDS)
            for r, (qa, nq) in enumerate(ROUNDS):
                last = r == NR - 1
                kv_filler = kv_deferred.pop(r, [])
                filler = []
                if r == 0:
                    filler += [t_wo]
                for rr in WO_SCHED.get(r, []):
                    filler += wo_thunks(*ROUNDS[rr])
                if r == 0:
                    qn, nn = ROUNDS[1]
                    filler += load_x_thunk(xkt, qn, nn, holder, ("xk", 1))
                    filler += load_x_thunk(xvt, qn, nn, holder, ("xv", 1))
                    filler += load_x_thunk(xqt, qn, nn, holder, ("xq", 1))
                if r + 2 < NR:
                    qn2, nn2 = ROUNDS[r + 2]
                    filler += load_x_thunk(xkt, qn2, nn2, holder, ("xk", r + 2))
                    filler += load_x_thunk(xvt, qn2, nn2, holder, ("xv", r + 2))
                    filler += load_x_thunk(xqt, qn2, nn2, holder, ("xq", r + 2))
                if r + 1 < NR:
                    qn, nn = ROUNDS[r + 1]
                    filler += proj_qk_thunks(
                        qn, nn, holder, ("xq", r + 1), wq_sb, bq_sb, qhT
                    )
                    filler += proj_qk_thunks(
                        qn, nn, holder, ("xk", r + 1), wk_sb, bk_sb, khT
                    )
                    kv_deferred[r + 1] = proj_v_thunks(qn, nn, holder, ("xv", r + 1))

                ats0, ats1 = {}, {}
                jl0 = attn_jloop_thunks(qa, nq, 0, ats0)
                nm0 = attn_norm_thunks(qa, nq, 0, ats0)
                jl1 = attn_jloop_thunks(qa, nq, 1, ats1)
                cbase = qa // P
                if not last:
                    nm1 = attn_norm_thunks(qa, nq, 1, ats1)
                    att = jl0 + nm0 + jl1 + nm1
                    seg1, seg2 = att[:cbase], att[cbase:]
                    emit_interleaved(seg1, kv_filler)
                    cut = (2 * len(filler)) // 3
                    emit_interleaved(seg2[:-8], filler[:cut])
                    emit_interleaved(seg2[-8:], filler[cut:])
                else:
                    rb_holder = {}
                    act_part, mul_part = attn_norm_fast_thunks(
                        qa, nq, 1, ats1, rb_holder
                    )
                    wo_pre, wo_post = wo_tail_thunks(qa, nq)
                    att = jl0 + nm0 + jl1
                    seg1, seg2 = att[:cbase], att[cbase:]
                    emit_interleaved(seg1, kv_filler)
                    # hold one earlier-round wo unit back as PE filler for the
                    # final normalization window (it only reads old atT)
                    tail_fill, body = filler[-1:], filler[:-1]
                    emit_interleaved(seg2, body)
                    for t in act_part:
                        t()
                    for t in tail_fill:
                        t()
                    for t in wo_pre:
                        t()
                    for t in mul_part:
                        t()
                    for t in wo_post:
                        t()

    nc.compile()
    return nc


def _get_nc():
    if "nc" not in _CACHE:
        _CACHE["nc"] = _build()
    return _CACHE["nc"]


def _tile_x(x):
    # [S, E] -> x.T [(kt p), s] -> [qc, p, kt, nq] (contiguous per partition)
    a = np.ascontiguousarray(x.T).astype(np.float16)
    return np.ascontiguousarray(
        a.reshape(KT, P, S // NQMAX, NQMAX).transpose(2, 1, 0, 3)
    )


def _tile_w(w):
    # [(kt p), m] -> [p, kt, m] (contiguous per partition)
    kt = w.shape[0] // P
    a = np.ascontiguousarray(w).astype(np.float16)
    return np.ascontiguousarray(a.reshape(kt, P, w.shape[1]).transpose(1, 0, 2))


def make_in_maps(q, k, v, Wq, bq, Wk, bk, Wv, bv, Wo):
    in_maps = []
    for core in range(8):
        b, g = divmod(core, 4)
        cs = slice(g * CW, (g + 1) * CW)
        in_maps.append(
            {
                "xqt": _tile_x(q[b]),
                "xkt": _tile_x(k[b]),
                "xvt": _tile_x(v[b]),
                "wq": _tile_w(Wq[:, cs]),
                "wk": _tile_w(Wk[:, cs]),
                "wv": _tile_w(Wv[:, cs]),
                "wo": _tile_w(Wo[cs, :]),
                "bq": np.ascontiguousarray(bq[cs]),
                "bk": np.ascontiguousarray(bk[cs]),
                "bv": np.ascontiguousarray(bv[cs]).astype(np.float16),
            }
        )
    return in_maps


def run(q, k, v, Wq, bq, Wk, bk, Wv, bv, Wo, bo, **run_kwargs):
    """Returns (output, BassKernelResults)."""
    from concourse.bass_utils import run_bass_kernel_spmd

    q, k, v = (np.asarray(x, np.float32) for x in (q, k, v))
    nc = _get_nc()
    in_maps = make_in_maps(
        q, k, v,
        np.asarray(Wq, np.float32), np.asarray(bq, np.float32),
        np.asarray(Wk, np.float32), np.asarray(bk, np.float32),
        np.asarray(Wv, np.float32), np.asarray(bv, np.float32),
        np.asarray(Wo, np.float32),
    )
    res = run_bass_kernel_spmd(nc, in_maps, list(range(8)), **run_kwargs)
    out = np.zeros((B, S, E), np.float32)
    for core in range(8):
        out[core // 4] += res.results[core]["out"].astype(np.float32)
    out += np.asarray(bo, np.float32)[None, None, :]
    return out, res


def kernel(q, k, v, Wq, bq, Wk, bk, Wv, bv, Wo, bo):
    return run(q, k, v, Wq, bq, Wk, bk, Wv, bv, Wo, bo)[0]


# revision 39
# speedup vs baseline: 1.1357x; 1.1357x over previous
"""Trainium2 Bass kernel for causal MultiHeadAttention (B=2, S=2048, E=1024, H=16).

Sharding: 8 cores = 2 (batch) x 4 (head groups of 4, Megatron-style).
Each core computes, for its batch b and head group g:
  - Q/K projections into transposed layout qhT/khT [256, S]  (256 = 4 heads x 64)
  - V projection into natural layout vh [S, 256] with a ones-column per head
  - causal attention with scores kept transposed [k, q]; softmax denominators
    come out of the PV matmul via the ones-column; no max-subtraction needed
    (|scores/sqrt(D)| <~ 6 so exp is well within fp32 range; masked entries are
    zeroed AFTER exp, which matches the reference's -1e9 masking exactly)
  - partial output projection attn_concat @ Wo[rows of g]  -> [S, E]
Host sums the 4 partials per batch and adds bo.

All matmul operands are float16 (full PE rate, fp32 PSUM accumulation).
Schedule notes:
  - Prologue input DMAs are split across the two HWDGE queues (SP and ACT;
    each dispatch costs ~700ns of serial queue time, and ACT is idle during
    the prologue).  Steady-state loads stay on SP; the final output DMAs
    alternate queues again.
  - Causal masks are built on-device (memset + affine_select), ones tiles
    via memset: no mask/ones DMA.
  - x and weights arrive PRE-TILED from the host ([qc, p, kt, nq] / [p, kt, m])
    so every input DMA is contiguous per partition: 8KB descriptors instead
    of 1KB (x) / 512B (weights) strided rows — measured ~2us faster.
  - x chunks are prefetched TWO rounds ahead (xpool bufs=9) so projection
    matmuls never wait on HBM arrivals mid-kernel.
  - No PE warmup at t~0: warmup matmuls measured ~1.5us SLOWER (they feed
    the HAM power governor, which then clamps rounds 0-1 to half rate).
  - Prologue projection matmuls are emitted kt-major (m inner) so a freshly
    arriving x-tile is consumed by both m-blocks back to back, halving the
    HBM rate the prologue needs.
  - Q rounds are 4 x 512 (splitting rounds was measured ~6-10us slower:
    every extra round re-loads all prior k-block stationaries, ~400ns fixed
    PE cost per (head-pair, k-block) unit).
  - Output tiles are [128, 1024] per row-block: two casts + ONE output DMA
    (halves the ~600ns/dispatch load on the SP queue).
  - Final-round tail: wo kt0 matmuls are emitted BEFORE the last head-pair's
    normalization thunks (PE-queue order is execution order, so emitting
    them later serializes them behind the norm), the 1/denominator
    broadcast is a rank-1 PE matmul into a borrowed scores-psum bank
    instead of a gpsimd partition_broadcast, and the final psum->sbuf casts
    alternate between DVE and ACT.
"""

import numpy as np

B, S, E, H = 2, 2048, 1024, 16
D = E // H            # 64 head dim
HL = 4                # heads per core
CW = HL * D           # 256 local channels
P = 128
KT = E // P           # 8 contraction tiles for the input projections
D1 = D + 1            # head slot in vh (+ ones column)
NQMAX = 512

ROUNDS = [(0, 512), (512, 512), (1024, 512), (1536, 512)]
# round r -> earlier rounds whose output projection runs as its filler
WO_SCHED = {2: [0], 3: [1, 2]}

_CACHE = {}


def _pin_act_table(mybir, bacc):
    """Force all activations onto one LUT set containing exp+ln+identity, so
    the ACT engine never reloads tables mid-kernel (1.3us per reload)."""
    from concourse.hw_specs import get_activation_tables

    need = {
        mybir.ActivationFunctionType.Exp,
        mybir.ActivationFunctionType.Ln,
        mybir.ActivationFunctionType.Identity,
    }
    orig = get_activation_tables("gen3")
    target = next(n for n, fs in orig.items() if need <= fs)
    pinned = {n: (fs if n == target else set()) for n, fs in orig.items()}
    bacc.get_activation_tables = lambda arch: pinned


def _build(num_devices=8):
    import concourse.mybir as mybir
    import concourse.tile as tile
    from concourse import bacc

    _pin_act_table(mybir, bacc)

    f32 = mybir.dt.float32
    h16 = mybir.dt.float16
    Identity = mybir.ActivationFunctionType.Identity
    Ln = mybir.ActivationFunctionType.Ln
    Exp = mybir.ActivationFunctionType.Exp

    nc = bacc.Bacc(
        "TRN2", target_bir_lowering=False, debug=False, num_devices=num_devices
    )

    def din(name, shape, dt=f32):
        return nc.dram_tensor(name, list(shape), dt, kind="ExternalInput").ap()

    # x and weights arrive pre-tiled from the host so every DMA is
    # contiguous per partition (8KB descriptors instead of 1KB/512B rows)
    QC = S // NQMAX
    xqt = din("xqt", (QC, P, KT, NQMAX), h16)
    xkt = din("xkt", (QC, P, KT, NQMAX), h16)
    xvt = din("xvt", (QC, P, KT, NQMAX), h16)
    wq = din("wq", (P, KT, CW), h16)
    wk = din("wk", (P, KT, CW), h16)
    wv = din("wv", (P, KT, CW), h16)
    wo = din("wo", (P, CW // P, E), h16)
    bq = din("bq", (CW,))
    bk = din("bk", (CW,))
    bv = din("bv", (CW,), h16)
    out = nc.dram_tensor("out", [S, E], h16, kind="ExternalOutput").ap()

    SB = S // P  # 16 k-blocks total

    with tile.TileContext(nc) as tc:
        with (
            tc.tile_pool(name="singles", bufs=1) as singles,
            tc.tile_pool(name="xpool", bufs=9) as xpool,
            tc.tile_pool(name="exp", bufs=10) as exp_pool,
            tc.tile_pool(name="outp", bufs=4) as out_pool,
            tc.tile_pool(name="small", bufs=4) as small_pool,
            tc.tile_pool(name="stage", bufs=6) as stage_pool,
            tc.tile_pool(name="proj_ps", bufs=2, space="PSUM") as proj_ps,
            tc.tile_pool(name="scores_ps", bufs=2, space="PSUM") as scores_ps,
            tc.tile_pool(name="attn_ps", bufs=2, space="PSUM") as attn_ps,
        ):
            dma = nc.sync.dma_start       # steady-state loads
            dma2 = nc.scalar.dma_start    # second HWDGE queue: prologue only
            dma_out = nc.sync.dma_start

            # --- persistent SBUF tensors -------------------------------------
            wq_sb = singles.tile([P, KT, CW], h16, tag="wq")
            wk_sb = singles.tile([P, KT, CW], h16, tag="wk")
            wv_sb = singles.tile([P, KT, CW], h16, tag="wv")
            wo_sb = singles.tile([P, CW // P, E], h16, tag="wo")
            masks_sb = singles.tile([P, 4, NQMAX], h16, tag="masks")
            bq_sb = singles.tile([P, 2], f32, tag="bq")
            bk_sb = singles.tile([P, 2], f32, tag="bk")
            bv_row = singles.tile([1, CW], h16, tag="bv")
            ones_col = singles.tile([1, P], h16, tag="ones")

            qhT = [singles.tile([P, S], h16, name=f"qhT{m}", tag=f"qhT{m}") for m in range(2)]
            khT = [singles.tile([P, S], h16, name=f"khT{m}", tag=f"khT{m}") for m in range(2)]
            atT = [singles.tile([P, S], h16, name=f"atT{m}", tag=f"atT{m}") for m in range(2)]
            vh = singles.tile([P, SB, HL, D1], h16, tag="vh")

            def t_consts():
                # no DMA dependencies: runs at t=0 on otherwise-idle engines
                nc.vector.memset(ones_col, 1.0)
                nc.vector.memset(vh[:, :, :, D:D1], 1.0)
                # causal masks on-device: masks_sb[p, jj, q] = (q >= p + 128*jj)
                nc.vector.memset(masks_sb, 1.0)
                nc.gpsimd.affine_select(
                    out=masks_sb,
                    in_=masks_sb,
                    pattern=[[-P, 4], [1, NQMAX]],
                    channel_multiplier=-1,
                    base=0,
                    compare_op=mybir.AluOpType.is_ge,
                    fill=0.0,
                )

            def t_wk():
                dma(out=wk_sb[:, :1, :], in_=wk[:, :1, :])
                dma(out=wk_sb[:, 1 : KT // 2, :], in_=wk[:, 1 : KT // 2, :])
                dma(out=wk_sb[:, KT // 2 :, :], in_=wk[:, KT // 2 :, :])
                dma(out=bk_sb, in_=bk.rearrange("(m p) -> p m", p=P))

            def t_wv():
                dma2(out=wv_sb, in_=wv)
                dma2(out=bv_row, in_=bv.unsqueeze(0))

            def t_wq():
                dma2(out=wq_sb, in_=wq)
                dma2(out=bq_sb, in_=bq.rearrange("(m p) -> p m", p=P))

            def t_wo():
                dma(out=wo_sb, in_=wo)

            # --- stage helpers (thunk-list builders) -------------------------
            def load_x_thunk(src, qa, nq, holder, key, eng=None):
                def t():
                    d = eng or dma
                    tl = xpool.tile([P, KT, NQMAX], h16, name="xchunk", tag="xchunk")
                    rsrc = src[qa // NQMAX]
                    h = KT // 2
                    if eng is not None:
                        # prologue: fine slices so the first matmuls start early
                        d(out=tl[:, :1, :nq], in_=rsrc[:, :1, :nq])
                        d(out=tl[:, 1:h, :nq], in_=rsrc[:, 1:h, :nq])
                        d(out=tl[:, h:, :nq], in_=rsrc[:, h:, :nq])
                    else:
                        # prefetched 2 rounds ahead: latency-insensitive, one
                        # fully contiguous DMA (halves SP dispatch load)
                        d(out=tl[:, :, :nq], in_=rsrc[:, :, :nq])
                    holder[key] = tl
                return [t]

            def proj_qk_thunks(qa, nq, holder, key, w_sb, b_sb, dstT, kt_major=False):
                """kt_major: emit (kt0,m0),(kt0,m1),(kt1,m0)... so each x-tile
                is consumed by both m-blocks back-to-back (halves the HBM rate
                the prologue needs)."""
                thunks = []
                pss = {}
                def mk_mm(m, kt):
                    def t():
                        if kt == 0:
                            pss[m] = proj_ps.tile([P, NQMAX], f32, name="proj", tag="proj")
                        nc.tensor.matmul(
                            pss[m][:, :nq],
                            w_sb[:, kt, m * P : (m + 1) * P],
                            holder[key][:, kt, :nq],
                            start=(kt == 0),
                            stop=(kt == KT - 1),
                        )
                    return t
                def mk_copy(m):
                    def t():
                        nc.vector.tensor_scalar_add(
                            out=dstT[m][:, qa : qa + nq],
                            in0=pss[m][:, :nq],
                            scalar1=b_sb[:, m : m + 1],
                        )
                    return t
                if kt_major:
                    for kt in range(KT):
                        for m in range(2):
                            thunks.append(mk_mm(m, kt))
                    thunks.append(mk_copy(0))
                    thunks.append(mk_copy(1))
                else:
                    for m in range(2):
                        for kt in range(KT):
                            thunks.append(mk_mm(m, kt))
                        thunks.append(mk_copy(m))
                return thunks

            def proj_v_thunks(qa, nq, holder, key):
                thunks = []
                pss = {}
                for mb in range(nq // P):
                    j = qa // P + mb
                    def mk_mm(mb, kt):
                        def t():
                            if kt == 0:
                                pss[mb] = proj_ps.tile([P, NQMAX], f32, name="proj", tag="proj")
                            nc.tensor.matmul(
                                pss[mb][:, :CW],
                                holder[key][:, kt, mb * P : (mb + 1) * P],
                                wv_sb[:, kt, :],
                                start=(kt == 0),
                                stop=False,
                            )
                        return t
                    for kt in range(KT):
                        thunks.append(mk_mm(mb, kt))
                    def mk_tail(mb, j):
                        def t():
                            nc.tensor.matmul(
                                pss[mb][:, :CW],
                                ones_col,
                                bv_row,
                                start=False,
                                stop=True,
                            )
                            nc.vector.tensor_copy(
                                out=vh[:, j, :, 0:D],
                                in_=pss[mb][:, :CW].rearrange("p (h d) -> p h d", h=HL),
                            )
                        return t
                    thunks.append(mk_tail(mb, j))
                return thunks

            def attn_jloop_thunks(qa, nq, hp, ats):
                thunks = []
                cbase = qa // P
                nblk = (qa + nq) // P
                scale = float(1.0 / np.sqrt(D))
                def mk_j(j):
                    def t():
                        if j == 0:
                            ats[0] = attn_ps.tile([D1, NQMAX], f32, name="attn", tag="attn")
                            ats[1] = attn_ps.tile([D1, NQMAX], f32, name="attn", tag="attn")
                        jj = j - cbase
                        q0 = jj * P if jj > 0 else 0
                        sc2 = scores_ps.tile([P, 2, NQMAX], f32, name="sc2", tag="sc2")
                        for hh in range(2):
                            po = hh * D
                            nc.tensor.matmul(
                                sc2[:, hh, q0:nq],
                                khT[hp][po : po + D, j * P : (j + 1) * P],
                                qhT[hp][po : po + D, qa + q0 : qa + nq],
                                start=True,
                                stop=True,
                            )
                        ex2 = exp_pool.tile([P, 2, NQMAX], h16, name="ex2", tag="ex2")
                        nc.scalar.activation(
                            out=ex2[:, :, q0:nq], in_=sc2[:, :, q0:nq], func=Exp,
                            scale=scale,
                        )
                        if jj >= 0:
                            for hh in range(2):
                                exh = ex2[:, hh, q0:nq]
                                nc.vector.tensor_mul(exh, exh, masks_sb[:, jj, q0:nq])
                        for hh in range(2):
                            nc.tensor.matmul(
                                ats[hh][:, q0:nq],
                                vh[:, j, 2 * hp + hh, :],
                                ex2[:, hh, q0:nq],
                                start=(j == 0),
                                stop=(j == nblk - 1),
                            )
                    return t
                for j in range(nblk):
                    thunks.append(mk_j(j))
                return thunks

            def attn_norm_thunks(qa, nq, hp, ats):
                """Standard (mid-kernel) normalization: ln -> copy-out ->
                exp(-x) -> gpsimd broadcast -> multiply into atT."""
                thunks = []
                atu = {}
                lns = {}
                def mk_stage(hh):
                    def t():
                        ls = small_pool.tile([1, NQMAX], f32, name="ls", tag="ls")
                        nc.scalar.activation(
                            out=ls[:, :nq], in_=ats[hh][D : D + 1, :nq], func=Ln,
                            scale=1.0,
                        )
                        lns[hh] = ls
                        atu[hh] = stage_pool.tile([D, NQMAX], h16, name="atu", tag="atu")
                        nc.vector.tensor_copy(atu[hh][:, :nq], ats[hh][0:D, :nq])
                    return t
                def mk_norm(hh):
                    def t():
                        po = hh * D
                        rs = small_pool.tile([1, NQMAX], f32, name="rs", tag="rs")
                        nc.scalar.activation(
                            out=rs[:, :nq], in_=lns[hh][:, :nq], func=Exp, scale=-1.0
                        )
                        rb = small_pool.tile([D, NQMAX], f32, name="rb", tag="rb")
                        nc.gpsimd.partition_broadcast(rb[:, :nq], rs[:, :nq])
                        nc.vector.tensor_mul(
                            atT[hp][po : po + D, qa : qa + nq],
                            atu[hh][:, :nq],
                            rb[:, :nq],
                        )
                    return t
                thunks.append(mk_stage(0))
                thunks.append(mk_stage(1))
                thunks.append(mk_norm(0))
                thunks.append(mk_norm(1))
                return thunks

            def attn_norm_fast_thunks(qa, nq, hp, ats, rb_ps_holder):
                """Final-round normalization for the last head pair: ACT chain
                ordered ln0,exp0,ln1,exp1; 1/denominator broadcast via a
                rank-1 PE matmul into a borrowed scores bank; staging copies
                run on DVE in parallel with the ACT chain."""
                thunks = []
                lns = {}
                rss = {}
                atu = {}
                def mk_ln(hh):
                    def t():
                        ls = small_pool.tile([1, NQMAX], f32, name="ls", tag="ls")
                        nc.scalar.activation(
                            out=ls[:, :nq], in_=ats[hh][D : D + 1, :nq], func=Ln,
                            scale=1.0,
                        )
                        lns[hh] = ls
                        atu[hh] = stage_pool.tile([D, NQMAX], h16, name="atu", tag="atu")
                        nc.vector.tensor_copy(atu[hh][:, :nq], ats[hh][0:D, :nq])
                    return t
                def mk_exp(hh):
                    def t():
                        rs = small_pool.tile([1, NQMAX], h16, name="rs16", tag="rs16")
                        nc.scalar.activation(
                            out=rs[:, :nq], in_=lns[hh][:, :nq], func=Exp, scale=-1.0
                        )
                        rss[hh] = rs
                    return t
                def mk_rb(hh):
                    def t():
                        if hh == 0:
                            rb_ps_holder[0] = scores_ps.tile(
                                [P, 2, NQMAX], f32, name="sc2", tag="sc2"
                            )
                        nc.tensor.matmul(
                            rb_ps_holder[0][0:D, hh, :nq],
                            ones_col[0:1, 0:D],
                            rss[hh][:, :nq],
                            start=True,
                            stop=True,
                        )
                    return t
                def mk_mul(hh):
                    def t():
                        po = hh * D
                        nc.vector.tensor_mul(
                            atT[hp][po : po + D, qa : qa + nq],
                            atu[hh][:, :nq],
                            rb_ps_holder[0][0:D, hh, :nq],
                        )
                    return t
                # ACT chain first (immediately runnable), PE/DVE parts later
                # so PE filler emitted in between does not serialize the chain
                act_part = [mk_ln(0), mk_exp(0), mk_ln(1), mk_exp(1)]
                mul_part = [mk_rb(0), mk_mul(0), mk_rb(1), mk_mul(1)]
                return act_part, mul_part

            def wo_thunks(qa, nq):
                """Output projection per 128-row block ms: two [P, NQMAX]
                psum halves, two casts, ONE [P, E] output DMA."""
                thunks = []
                for mb in range(nq // P):
                    ms = qa // P + mb
                    def mk(ms):
                        def t():
                            ot = out_pool.tile([P, E], h16, name="ot", tag="ot")
                            for n in range(2):
                                ps = proj_ps.tile([P, NQMAX], f32, name="proj", tag="proj")
                                for kt in range(CW // P):
                                    nc.tensor.matmul(
                                        ps,
                                        atT[kt][:, ms * P : (ms + 1) * P],
                                        wo_sb[:, kt, n * NQMAX : (n + 1) * NQMAX],
                                        start=(kt == 0),
                                        stop=(kt == CW // P - 1),
                                    )
                                nc.vector.tensor_copy(
                                    ot[:, n * NQMAX : (n + 1) * NQMAX], ps
                                )
                            dma_out(out=out[ms * P : (ms + 1) * P, :], in_=ot)
                        return t
                    thunks.append(mk(ms))
                return thunks

            def wo_tail_thunks(qa, nq):
                """Final-round wo as (pre, post): `pre` = kt0 matmuls of the
                first wave (only need atT[0]); `post` = the rest.  Casts
                alternate DVE/ACT; one DMA per 128-row block."""
                mss = [qa // P + mb for mb in range(nq // P)]
                pre, post = [], []
                pss = {}
                ots = {}
                sc_shared = {}
                def mk_kt0(u, ms, n):
                    def t():
                        i = 2 * u + n
                        if i % 4 < 2:
                            pss[i] = proj_ps.tile([P, NQMAX], f32, name="proj", tag="proj")
                        else:
                            if i % 4 == 2:
                                sc_shared[i // 4] = scores_ps.tile(
                                    [P, 2, NQMAX], f32, name="sc2", tag="sc2"
                                )
                            pss[i] = sc_shared[i // 4][:, i % 2, :]
                        nc.tensor.matmul(
                            pss[i],
                            atT[0][:, ms * P : (ms + 1) * P],
                            wo_sb[:, 0, n * NQMAX : (n + 1) * NQMAX],
                            start=True,
                            stop=False,
                        )
                    return t
                def mk_kt1(u, ms, n):
                    def t():
                        i = 2 * u + n
                        if n == 0:
                            ots[u] = out_pool.tile([P, E], h16, name="ot", tag="ot")
                        nc.tensor.matmul(
                            pss[i],
                            atT[1][:, ms * P : (ms + 1) * P],
                            wo_sb[:, 1, n * NQMAX : (n + 1) * NQMAX],
                            start=False,
                            stop=True,
                        )
                        dst = ots[u][:, n * NQMAX : (n + 1) * NQMAX]
                        if n == 0:
                            nc.vector.tensor_copy(dst, pss[i])
                        else:
                            nc.scalar.activation(
                                out=dst, in_=pss[i], func=Identity, scale=1.0
                            )
                            # alternate the final output dispatches across the
                            # two HWDGE queues (~600ns of queue time each)
                            eng = dma_out if u % 2 == 0 else dma2
                            eng(out=out[ms * P : (ms + 1) * P, :], in_=ots[u])
                    return t
                # wave A: units 0,1 (4 psum banks); wave B: units 2,3
                for u, ms in enumerate(mss[:2]):
                    for n in range(2):
                        pre.append(mk_kt0(u, ms, n))
                for u, ms in enumerate(mss[:2]):
                    for n in range(2):
                        post.append(mk_kt1(u, ms, n))
                for u, ms in enumerate(mss[2:], start=2):
                    for n in range(2):
                        post.append(mk_kt0(u, ms, n))
                for u, ms in enumerate(mss[2:], start=2):
                    for n in range(2):
                        post.append(mk_kt1(u, ms, n))
                return pre, post

            def emit_interleaved(primary, filler):
                fi = 0
                n = max(len(primary), 1)
                f = len(filler)
                for i, t in enumerate(primary):
                    t()
                    while fi * n < f * (i + 1):
                        filler[fi]()
                        fi += 1
                for t in filler[fi:]:
                    t()

            # --- main schedule ----------------------------------------------
            holder = {}
            qa0, nq0 = ROUNDS[0]
            prologue = (
                [t_consts, t_wk]
                + load_x_thunk(xkt, qa0, nq0, holder, ("xk", 0), eng=dma2)
                + proj_qk_thunks(qa0, nq0, holder, ("xk", 0), wk_sb, bk_sb, khT, kt_major=True)
                + [t_wv]
                + load_x_thunk(xvt, qa0, nq0, holder, ("xv", 0), eng=dma)
                + proj_v_thunks(qa0, nq0, holder, ("xv", 0))
                + [t_wq]
                + load_x_thunk(xqt, qa0, nq0, holder, ("xq", 0), eng=dma2)
                + proj_qk_thunks(qa0, nq0, holder, ("xq", 0), wq_sb, bq_sb, qhT, kt_major=True)
            )
            for t in prologue:
                t()
            kv_deferred = {}
            NR = len(ROUNDS)
            for r, (qa, nq) in enumerate(ROUNDS):
                last = r == NR - 1
                kv_filler = kv_deferred.pop(r, [])
                filler = []
                if r == 0:
                    filler += [t_wo]
                for rr in WO_SCHED.get(r, []):
                    filler += wo_thunks(*ROUNDS[rr])
                if r == 0:
                    qn, nn = ROUNDS[1]
                    filler += load_x_thunk(xkt, qn, nn, holder, ("xk", 1))
                    filler += load_x_thunk(xvt, qn, nn, holder, ("xv", 1))
                    filler += load_x_thunk(xqt, qn, nn, holder, ("xq", 1))
                if r + 2 < NR:
                    qn2, nn2 = ROUNDS[r + 2]
                    filler += load_x_thunk(xkt, qn2, nn2, holder, ("xk", r + 2))
                    filler += load_x_thunk(xvt, qn2, nn2, holder, ("xv", r + 2))
                    filler += load_x_thunk(xqt, qn2, nn2, holder, ("xq", r + 2))
                if r + 1 < NR:
                    qn, nn = ROUNDS[r + 1]
                    filler += proj_qk_thunks(
                        qn, nn, holder, ("xq", r + 1), wq_sb, bq_sb, qhT
                    )
                    filler += proj_qk_thunks(
                        qn, nn, holder, ("xk", r + 1), wk_sb, bk_sb, khT
                    )
                    kv_deferred[r + 1] = proj_v_thunks(qn, nn, holder, ("xv", r + 1))

                ats0, ats1 = {}, {}
                jl0 = attn_jloop_thunks(qa, nq, 0, ats0)
                nm0 = attn_norm_thunks(qa, nq, 0, ats0)
                jl1 = attn_jloop_thunks(qa, nq, 1, ats1)
                cbase = qa // P
                if not last:
                    nm1 = attn_norm_thunks(qa, nq, 1, ats1)
                    att = jl0 + nm0 + jl1 + nm1
                    seg1, seg2 = att[:cbase], att[cbase:]
                    emit_interleaved(seg1, kv_filler)
                    cut = (2 * len(filler)) // 3
                    emit_interleaved(seg2[:-8], filler[:cut])
                    emit_interleaved(seg2[-8:], filler[cut:])
                else:
                    rb_holder = {}
                    act_part, mul_part = attn_norm_fast_thunks(
                        qa, nq, 1, ats1, rb_holder
                    )
                    wo_pre, wo_post = wo_tail_thunks(qa, nq)
                    att = jl0 + nm0 + jl1
                    seg1, seg2 = att[:cbase], att[cbase:]
                    emit_interleaved(seg1, kv_filler)
                    # hold one earlier-round wo unit back as PE filler for the
                    # final normalization window (it only reads old atT)
                    tail_fill, body = filler[-1:], filler[:-1]
                    emit_interleaved(seg2, body)
                    for t in act_part:
                        t()
                    for t in tail_fill:
                        t()
                    for t in wo_pre:
                        t()
                    for t in mul_part:
                        t()
                    for t in wo_post:
                        t()

    nc.compile()
    return nc


def _get_nc():
    if "nc" not in _CACHE:
        _CACHE["nc"] = _build()
    return _CACHE["nc"]


def _tile_x(x):
    # [S, E] -> x.T [(kt p), s] -> [qc, p, kt, nq] (contiguous per partition)
    a = np.ascontiguousarray(x.T).astype(np.float16)
    return np.ascontiguousarray(
        a.reshape(KT, P, S // NQMAX, NQMAX).transpose(2, 1, 0, 3)
    )


def _tile_w(w):
    # [(kt p), m] -> [p, kt, m] (contiguous per partition)
    kt = w.shape[0] // P
    a = np.ascontiguousarray(w).astype(np.float16)
    return np.ascontiguousarray(a.reshape(kt, P, w.shape[1]).transpose(1, 0, 2))


def make_in_maps(q, k, v, Wq, bq, Wk, bk, Wv, bv, Wo):
    in_maps = []
    for core in range(8):
        b, g = divmod(core, 4)
        cs = slice(g * CW, (g + 1) * CW)
        in_maps.append(
            {
                "xqt": _tile_x(q[b]),
                "xkt": _tile_x(k[b]),
                "xvt": _tile_x(v[b]),
                "wq": _tile_w(Wq[:, cs]),
                "wk": _tile_w(Wk[:, cs]),
                "wv": _tile_w(Wv[:, cs]),
                "wo": _tile_w(Wo[cs, :]),
                "bq": np.ascontiguousarray(bq[cs]),
                "bk": np.ascontiguousarray(bk[cs]),
                "bv": np.ascontiguousarray(bv[cs]).astype(np.float16),
            }
        )
    return in_maps


def run(q, k, v, Wq, bq, Wk, bk, Wv, bv, Wo, bo, **run_kwargs):
    """Returns (output, BassKernelResults)."""
    from concourse.bass_utils import run_bass_kernel_spmd

    q, k, v = (np.asarray(x, np.float32) for x in (q, k, v))
    nc = _get_nc()
    in_maps = make_in_maps(
        q, k, v,
        np.asarray(Wq, np.float32), np.asarray(bq, np.float32),
        np.asarray(Wk, np.float32), np.asarray(bk, np.float32),
        np.asarray(Wv, np.float32), np.asarray(bv, np.float32),
        np.asarray(Wo, np.float32),
    )
    res = run_bass_kernel_spmd(nc, in_maps, list(range(8)), **run_kwargs)
    out = np.zeros((B, S, E), np.float32)
    for core in range(8):
        out[core // 4] += res.results[core]["out"].astype(np.float32)
    out += np.asarray(bo, np.float32)[None, None, :]
    return out, res


def kernel(q, k, v, Wq, bq, Wk, bk, Wv, bv, Wo, bo):
    return run(q, k, v, Wq, bq, Wk, bk, Wv, bv, Wo, bo)[0]


# revision 40
# speedup vs baseline: 1.1607x; 1.0220x over previous
"""Trainium2 Bass kernel for causal MultiHeadAttention (B=2, S=2048, E=1024, H=16).

Sharding: 8 cores = 2 (batch) x 4 (head groups of 4, Megatron-style).
Each core computes, for its batch b and head group g:
  - Q/K projections into transposed layout qhT/khT [256, S]  (256 = 4 heads x 64)
  - V projection into natural layout vh [S, 256] with a ones-column per head
  - causal attention with scores kept transposed [k, q]; softmax denominators
    come out of the PV matmul via the ones-column; no max-subtraction needed
    (|scores/sqrt(D)| <~ 6 so exp is well within fp32 range; masked entries are
    zeroed AFTER exp, which matches the reference's -1e9 masking exactly)
  - partial output projection attn_concat @ Wo[rows of g]  -> [S, E]
Host sums the 4 partials per batch and adds bo.

All matmul operands are float16 (full PE rate, fp32 PSUM accumulation).
Schedule notes:
  - Prologue input DMAs are split across the two HWDGE queues (SP and ACT;
    each dispatch costs ~700ns of serial queue time, and ACT is idle during
    the prologue).  Steady-state loads stay on SP; the final output DMAs
    alternate queues again.
  - Causal masks are built on-device (memset + affine_select), ones tiles
    via memset: no mask/ones DMA.
  - x and weights arrive PRE-TILED from the host ([qc, p, kt, nq] / [p, kt, m])
    so every input DMA is contiguous per partition: 8KB descriptors instead
    of 1KB (x) / 512B (weights) strided rows — measured ~2us faster.
  - x chunks are prefetched TWO rounds ahead (xpool bufs=9) so projection
    matmuls never wait on HBM arrivals mid-kernel.
  - No PE warmup at t~0: warmup matmuls measured ~1.5us SLOWER (they feed
    the HAM power governor, which then clamps rounds 0-1 to half rate).
  - Prologue projection matmuls are emitted kt-major (m inner) so a freshly
    arriving x-tile is consumed by both m-blocks back to back, halving the
    HBM rate the prologue needs.
  - Q rounds are 4 x 512 (splitting rounds was measured ~6-10us slower:
    every extra round re-loads all prior k-block stationaries, ~400ns fixed
    PE cost per (head-pair, k-block) unit).
  - Output tiles are [128, 1024] per row-block: two casts + ONE output DMA
    (halves the ~600ns/dispatch load on the SP queue).
  - Final-round tail: wo kt0 matmuls are emitted BEFORE the last head-pair's
    normalization thunks (PE-queue order is execution order, so emitting
    them later serializes them behind the norm), the 1/denominator
    broadcast is a rank-1 PE matmul into a borrowed scores-psum bank
    instead of a gpsimd partition_broadcast, and the final psum->sbuf casts
    alternate between DVE and ACT.
"""

import numpy as np

B, S, E, H = 2, 2048, 1024, 16
D = E // H            # 64 head dim
HL = 4                # heads per core
CW = HL * D           # 256 local channels
P = 128
KT = E // P           # 8 contraction tiles for the input projections
D1 = D + 1            # head slot in vh (+ ones column)
NQMAX = 512

ROUNDS = [(0, 512), (512, 512), (1024, 512), (1536, 512)]
# round r -> earlier rounds whose output projection runs as its filler
WO_SCHED = {2: [0], 3: [1, 2]}

_CACHE = {}


def _pin_act_table(mybir, bacc):
    """Force all activations onto one LUT set containing exp+ln+identity, so
    the ACT engine never reloads tables mid-kernel (1.3us per reload)."""
    from concourse.hw_specs import get_activation_tables

    need = {
        mybir.ActivationFunctionType.Exp,
        mybir.ActivationFunctionType.Ln,
        mybir.ActivationFunctionType.Identity,
    }
    orig = get_activation_tables("gen3")
    target = next(n for n, fs in orig.items() if need <= fs)
    pinned = {n: (fs if n == target else set()) for n, fs in orig.items()}
    bacc.get_activation_tables = lambda arch: pinned


def _build(num_devices=8):
    import concourse.mybir as mybir
    import concourse.tile as tile
    from concourse import bacc

    _pin_act_table(mybir, bacc)

    f32 = mybir.dt.float32
    h16 = mybir.dt.float16
    Identity = mybir.ActivationFunctionType.Identity
    Ln = mybir.ActivationFunctionType.Ln
    Exp = mybir.ActivationFunctionType.Exp

    nc = bacc.Bacc(
        "TRN2", target_bir_lowering=False, debug=False, num_devices=num_devices
    )

    def din(name, shape, dt=f32):
        return nc.dram_tensor(name, list(shape), dt, kind="ExternalInput").ap()

    # x and weights arrive pre-tiled from the host so every DMA is
    # contiguous per partition (8KB descriptors instead of 1KB/512B rows)
    QC = S // NQMAX
    xqt = din("xqt", (QC, P, KT, NQMAX), h16)
    xkt = din("xkt", (QC, P, KT, NQMAX), h16)
    xvt = din("xvt", (QC, P, KT, NQMAX), h16)
    wq = din("wq", (P, KT, CW), h16)
    wk = din("wk", (P, KT, CW), h16)
    wv = din("wv", (P, KT, CW), h16)
    wo = din("wo", (P, CW // P, E), h16)
    bq = din("bq", (CW,))
    bk = din("bk", (CW,))
    bv = din("bv", (CW,), h16)
    out = nc.dram_tensor("out", [S, E], h16, kind="ExternalOutput").ap()

    SB = S // P  # 16 k-blocks total

    with tile.TileContext(nc) as tc:
        with (
            tc.tile_pool(name="singles", bufs=1) as singles,
            tc.tile_pool(name="xpool", bufs=9) as xpool,
            tc.tile_pool(name="exp", bufs=10) as exp_pool,
            tc.tile_pool(name="outp", bufs=4) as out_pool,
            tc.tile_pool(name="small", bufs=4) as small_pool,
            tc.tile_pool(name="stage", bufs=6) as stage_pool,
            tc.tile_pool(name="proj_ps", bufs=2, space="PSUM") as proj_ps,
            tc.tile_pool(name="scores_ps", bufs=2, space="PSUM") as scores_ps,
            tc.tile_pool(name="attn_ps", bufs=2, space="PSUM") as attn_ps,
        ):
            dma = nc.sync.dma_start       # steady-state loads
            dma2 = nc.scalar.dma_start    # second HWDGE queue: prologue only
            dma_out = nc.sync.dma_start

            # --- persistent SBUF tensors -------------------------------------
            wq_sb = singles.tile([P, KT, CW], h16, tag="wq")
            wk_sb = singles.tile([P, KT, CW], h16, tag="wk")
            wv_sb = singles.tile([P, KT, CW], h16, tag="wv")
            wo_sb = singles.tile([P, CW // P, E], h16, tag="wo")
            masks_sb = singles.tile([P, 4, NQMAX], h16, tag="masks")
            bq_sb = singles.tile([P, 2], f32, tag="bq")
            bk_sb = singles.tile([P, 2], f32, tag="bk")
            bv_row = singles.tile([1, CW], h16, tag="bv")
            ones_col = singles.tile([1, P], h16, tag="ones")

            qhT = [singles.tile([P, S], h16, name=f"qhT{m}", tag=f"qhT{m}") for m in range(2)]
            khT = [singles.tile([P, S], h16, name=f"khT{m}", tag=f"khT{m}") for m in range(2)]
            atT = [singles.tile([P, S], h16, name=f"atT{m}", tag=f"atT{m}") for m in range(2)]
            vh = singles.tile([P, SB, HL, D1], h16, tag="vh")

            def t_consts():
                # no DMA dependencies: runs at t=0 on otherwise-idle engines
                nc.vector.memset(ones_col, 1.0)
                nc.vector.memset(vh[:, :, :, D:D1], 1.0)
                # causal masks on-device: masks_sb[p, jj, q] = (q >= p + 128*jj)
                nc.vector.memset(masks_sb, 1.0)
                nc.gpsimd.affine_select(
                    out=masks_sb,
                    in_=masks_sb,
                    pattern=[[-P, 4], [1, NQMAX]],
                    channel_multiplier=-1,
                    base=0,
                    compare_op=mybir.AluOpType.is_ge,
                    fill=0.0,
                )

            def t_wk():
                dma(out=wk_sb[:, :1, :], in_=wk[:, :1, :])
                dma(out=wk_sb[:, 1 : KT // 2, :], in_=wk[:, 1 : KT // 2, :])
                dma(out=wk_sb[:, KT // 2 :, :], in_=wk[:, KT // 2 :, :])
                dma(out=bk_sb, in_=bk.rearrange("(m p) -> p m", p=P))

            def t_wv():
                dma2(out=wv_sb, in_=wv)
                dma2(out=bv_row, in_=bv.unsqueeze(0))

            def t_wq():
                dma2(out=wq_sb, in_=wq)
                dma2(out=bq_sb, in_=bq.rearrange("(m p) -> p m", p=P))

            def t_wo():
                dma(out=wo_sb, in_=wo)

            # --- stage helpers (thunk-list builders) -------------------------
            def load_x_thunk(src, qa, nq, holder, key, eng=None):
                def t():
                    d = eng or dma
                    tl = xpool.tile([P, KT, NQMAX], h16, name="xchunk", tag="xchunk")
                    rsrc = src[qa // NQMAX]
                    h = KT // 2
                    if eng is not None:
                        d(out=tl[:, :1, :nq], in_=rsrc[:, :1, :nq])
                        d(out=tl[:, 1:h, :nq], in_=rsrc[:, 1:h, :nq])
                    else:
                        d(out=tl[:, :h, :nq], in_=rsrc[:, :h, :nq])
                    d(out=tl[:, h:, :nq], in_=rsrc[:, h:, :nq])
                    holder[key] = tl
                return [t]

            def proj_qk_thunks(qa, nq, holder, key, w_sb, b_sb, dstT, kt_major=False):
                """kt_major: emit (kt0,m0),(kt0,m1),(kt1,m0)... so each x-tile
                is consumed by both m-blocks back-to-back (halves the HBM rate
                the prologue needs)."""
                thunks = []
                pss = {}
                def mk_mm(m, kt):
                    def t():
                        if kt == 0:
                            pss[m] = proj_ps.tile([P, NQMAX], f32, name="proj", tag="proj")
                        nc.tensor.matmul(
                            pss[m][:, :nq],
                            w_sb[:, kt, m * P : (m + 1) * P],
                            holder[key][:, kt, :nq],
                            start=(kt == 0),
                            stop=(kt == KT - 1),
                        )
                    return t
                def mk_copy(m):
                    def t():
                        nc.vector.tensor_scalar_add(
                            out=dstT[m][:, qa : qa + nq],
                            in0=pss[m][:, :nq],
                            scalar1=b_sb[:, m : m + 1],
                        )
                    return t
                if kt_major:
                    for kt in range(KT):
                        for m in range(2):
                            thunks.append(mk_mm(m, kt))
                    thunks.append(mk_copy(0))
                    thunks.append(mk_copy(1))
                else:
                    for m in range(2):
                        for kt in range(KT):
                            thunks.append(mk_mm(m, kt))
                        thunks.append(mk_copy(m))
                return thunks

            def proj_v_thunks(qa, nq, holder, key):
                thunks = []
                pss = {}
                for mb in range(nq // P):
                    j = qa // P + mb
                    def mk_mm(mb, kt):
                        def t():
                            if kt == 0:
                                pss[mb] = proj_ps.tile([P, NQMAX], f32, name="proj", tag="proj")
                            nc.tensor.matmul(
                                pss[mb][:, :CW],
                                holder[key][:, kt, mb * P : (mb + 1) * P],
                                wv_sb[:, kt, :],
                                start=(kt == 0),
                                stop=False,
                            )
                        return t
                    for kt in range(KT):
                        thunks.append(mk_mm(mb, kt))
                    def mk_tail(mb, j):
                        def t():
                            nc.tensor.matmul(
                                pss[mb][:, :CW],
                                ones_col,
                                bv_row,
                                start=False,
                                stop=True,
                            )
                            nc.vector.tensor_copy(
                                out=vh[:, j, :, 0:D],
                                in_=pss[mb][:, :CW].rearrange("p (h d) -> p h d", h=HL),
                            )
                        return t
                    thunks.append(mk_tail(mb, j))
                return thunks

            def attn_jloop_thunks(qa, nq, hp, ats):
                thunks = []
                cbase = qa // P
                nblk = (qa + nq) // P
                scale = float(1.0 / np.sqrt(D))
                def mk_j(j):
                    def t():
                        if j == 0:
                            ats[0] = attn_ps.tile([D1, NQMAX], f32, name="attn", tag="attn")
                            ats[1] = attn_ps.tile([D1, NQMAX], f32, name="attn", tag="attn")
                        jj = j - cbase
                        q0 = jj * P if jj > 0 else 0
                        sc2 = scores_ps.tile([P, 2, NQMAX], f32, name="sc2", tag="sc2")
                        for hh in range(2):
                            po = hh * D
                            nc.tensor.matmul(
                                sc2[:, hh, q0:nq],
                                khT[hp][po : po + D, j * P : (j + 1) * P],
                                qhT[hp][po : po + D, qa + q0 : qa + nq],
                                start=True,
                                stop=True,
                            )
                        ex2 = exp_pool.tile([P, 2, NQMAX], h16, name="ex2", tag="ex2")
                        nc.scalar.activation(
                            out=ex2[:, :, q0:nq], in_=sc2[:, :, q0:nq], func=Exp,
                            scale=scale,
                        )
                        if jj >= 0:
                            for hh in range(2):
                                exh = ex2[:, hh, q0:nq]
                                nc.vector.tensor_mul(exh, exh, masks_sb[:, jj, q0:nq])
                        for hh in range(2):
                            nc.tensor.matmul(
                                ats[hh][:, q0:nq],
                                vh[:, j, 2 * hp + hh, :],
                                ex2[:, hh, q0:nq],
                                start=(j == 0),
                                stop=(j == nblk - 1),
                            )
                    return t
                for j in range(nblk):
                    thunks.append(mk_j(j))
                return thunks

            def attn_norm_thunks(qa, nq, hp, ats):
                """Standard (mid-kernel) normalization: ln -> copy-out ->
                exp(-x) -> gpsimd broadcast -> multiply into atT."""
                thunks = []
                atu = {}
                lns = {}
                def mk_stage(hh):
                    def t():
                        ls = small_pool.tile([1, NQMAX], f32, name="ls", tag="ls")
                        nc.scalar.activation(
                            out=ls[:, :nq], in_=ats[hh][D : D + 1, :nq], func=Ln,
                            scale=1.0,
                        )
                        lns[hh] = ls
                        atu[hh] = stage_pool.tile([D, NQMAX], h16, name="atu", tag="atu")
                        nc.vector.tensor_copy(atu[hh][:, :nq], ats[hh][0:D, :nq])
                    return t
                def mk_norm(hh):
                    def t():
                        po = hh * D
                        rs = small_pool.tile([1, NQMAX], f32, name="rs", tag="rs")
                        nc.scalar.activation(
                            out=rs[:, :nq], in_=lns[hh][:, :nq], func=Exp, scale=-1.0
                        )
                        rb = small_pool.tile([D, NQMAX], f32, name="rb", tag="rb")
                        nc.gpsimd.partition_broadcast(rb[:, :nq], rs[:, :nq])
                        nc.vector.tensor_mul(
                            atT[hp][po : po + D, qa : qa + nq],
                            atu[hh][:, :nq],
                            rb[:, :nq],
                        )
                    return t
                thunks.append(mk_stage(0))
                thunks.append(mk_stage(1))
                thunks.append(mk_norm(0))
                thunks.append(mk_norm(1))
                return thunks

            def attn_norm_fast_thunks(qa, nq, hp, ats, rb_ps_holder):
                """Final-round normalization for the last head pair: ACT chain
                ordered ln0,exp0,ln1,exp1; 1/denominator broadcast via a
                rank-1 PE matmul into a borrowed scores bank; staging copies
                run on DVE in parallel with the ACT chain."""
                thunks = []
                lns = {}
                rss = {}
                atu = {}
                def mk_ln(hh):
                    def t():
                        ls = small_pool.tile([1, NQMAX], f32, name="ls", tag="ls")
                        nc.scalar.activation(
                            out=ls[:, :nq], in_=ats[hh][D : D + 1, :nq], func=Ln,
                            scale=1.0,
                        )
                        lns[hh] = ls
                        atu[hh] = stage_pool.tile([D, NQMAX], h16, name="atu", tag="atu")
                        nc.vector.tensor_copy(atu[hh][:, :nq], ats[hh][0:D, :nq])
                    return t
                def mk_exp(hh):
                    def t():
                        rs = small_pool.tile([1, NQMAX], h16, name="rs16", tag="rs16")
                        nc.scalar.activation(
                            out=rs[:, :nq], in_=lns[hh][:, :nq], func=Exp, scale=-1.0
                        )
                        rss[hh] = rs
                    return t
                def mk_rb(hh):
                    def t():
                        if hh == 0:
                            rb_ps_holder[0] = scores_ps.tile(
                                [P, 2, NQMAX], f32, name="sc2", tag="sc2"
                            )
                        nc.tensor.matmul(
                            rb_ps_holder[0][0:D, hh, :nq],
                            ones_col[0:1, 0:D],
                            rss[hh][:, :nq],
                            start=True,
                            stop=True,
                        )
                    return t
                def mk_mul(hh):
                    def t():
                        po = hh * D
                        nc.vector.tensor_mul(
                            atT[hp][po : po + D, qa : qa + nq],
                            atu[hh][:, :nq],
                            rb_ps_holder[0][0:D, hh, :nq],
                        )
                    return t
                # ACT chain first (immediately runnable), PE/DVE parts later
                # so PE filler emitted in between does not serialize the chain
                act_part = [mk_ln(0), mk_exp(0), mk_ln(1), mk_exp(1)]
                mul_part = [mk_rb(0), mk_mul(0), mk_rb(1), mk_mul(1)]
                return act_part, mul_part

            def wo_thunks(qa, nq):
                """Output projection per 128-row block ms: two [P, NQMAX]
                psum halves, two casts, ONE [P, E] output DMA."""
                thunks = []
                for mb in range(nq // P):
                    ms = qa // P + mb
                    def mk(ms):
                        def t():
                            ot = out_pool.tile([P, E], h16, name="ot", tag="ot")
                            for n in range(2):
                                ps = proj_ps.tile([P, NQMAX], f32, name="proj", tag="proj")
                                for kt in range(CW // P):
                                    nc.tensor.matmul(
                                        ps,
                                        atT[kt][:, ms * P : (ms + 1) * P],
                                        wo_sb[:, kt, n * NQMAX : (n + 1) * NQMAX],
                                        start=(kt == 0),
                                        stop=(kt == CW // P - 1),
                                    )
                                nc.vector.tensor_copy(
                                    ot[:, n * NQMAX : (n + 1) * NQMAX], ps
                                )
                            dma_out(out=out[ms * P : (ms + 1) * P, :], in_=ot)
                        return t
                    thunks.append(mk(ms))
                return thunks

            def wo_tail_thunks(qa, nq):
                """Final-round wo as (pre, post): `pre` = kt0 matmuls of the
                first wave (only need atT[0]); `post` = the rest.  Casts
                alternate DVE/ACT; one DMA per 128-row block."""
                mss = [qa // P + mb for mb in range(nq // P)]
                pre, post = [], []
                pss = {}
                ots = {}
                sc_shared = {}
                def mk_kt0(u, ms, n):
                    def t():
                        i = 2 * u + n
                        if i % 4 < 2:
                            pss[i] = proj_ps.tile([P, NQMAX], f32, name="proj", tag="proj")
                        else:
                            if i % 4 == 2:
                                sc_shared[i // 4] = scores_ps.tile(
                                    [P, 2, NQMAX], f32, name="sc2", tag="sc2"
                                )
                            pss[i] = sc_shared[i // 4][:, i % 2, :]
                        nc.tensor.matmul(
                            pss[i],
                            atT[0][:, ms * P : (ms + 1) * P],
                            wo_sb[:, 0, n * NQMAX : (n + 1) * NQMAX],
                            start=True,
                            stop=False,
                        )
                    return t
                def mk_kt1(u, ms, n):
                    def t():
                        i = 2 * u + n
                        if n == 0:
                            ots[u] = out_pool.tile([P, E], h16, name="ot", tag="ot")
                        nc.tensor.matmul(
                            pss[i],
                            atT[1][:, ms * P : (ms + 1) * P],
                            wo_sb[:, 1, n * NQMAX : (n + 1) * NQMAX],
                            start=False,
                            stop=True,
                        )
                        dst = ots[u][:, n * NQMAX : (n + 1) * NQMAX]
                        if n == 0:
                            nc.vector.tensor_copy(dst, pss[i])
                        else:
                            nc.scalar.activation(
                                out=dst, in_=pss[i], func=Identity, scale=1.0
                            )
                            # alternate the final output dispatches across the
                            # two HWDGE queues (~600ns of queue time each)
                            eng = dma_out if u % 2 == 0 else dma2
                            eng(out=out[ms * P : (ms + 1) * P, :], in_=ots[u])
                    return t
                # wave A: units 0,1 (4 psum banks); wave B: units 2,3
                for u, ms in enumerate(mss[:2]):
                    for n in range(2):
                        pre.append(mk_kt0(u, ms, n))
                for u, ms in enumerate(mss[:2]):
                    for n in range(2):
                        post.append(mk_kt1(u, ms, n))
                for u, ms in enumerate(mss[2:], start=2):
                    for n in range(2):
                        post.append(mk_kt0(u, ms, n))
                for u, ms in enumerate(mss[2:], start=2):
                    for n in range(2):
                        post.append(mk_kt1(u, ms, n))
                return pre, post

            def emit_interleaved(primary, filler):
                fi = 0
                n = max(len(primary), 1)
                f = len(filler)
                for i, t in enumerate(primary):
                    t()
                    while fi * n < f * (i + 1):
                        filler[fi]()
                        fi += 1
                for t in filler[fi:]:
                    t()

            # --- main schedule ----------------------------------------------
            holder = {}
            qa0, nq0 = ROUNDS[0]
            prologue = (
                [t_consts, t_wk]
                + load_x_thunk(xkt, qa0, nq0, holder, ("xk", 0), eng=dma2)
                + proj_qk_thunks(qa0, nq0, holder, ("xk", 0), wk_sb, bk_sb, khT, kt_major=True)
                + [t_wv]
                + load_x_thunk(xvt, qa0, nq0, holder, ("xv", 0), eng=dma)
                + proj_v_thunks(qa0, nq0, holder, ("xv", 0))
                + [t_wq]
                + load_x_thunk(xqt, qa0, nq0, holder, ("xq", 0), eng=dma2)
                + proj_qk_thunks(qa0, nq0, holder, ("xq", 0), wq_sb, bq_sb, qhT, kt_major=True)
            )
            for t in prologue:
                t()
            kv_deferred = {}
            NR = len(ROUNDS)
            for r, (qa, nq) in enumerate(ROUNDS):
                last = r == NR - 1
                kv_filler = kv_deferred.pop(r, [])
                filler = []
                if r == 0:
                    filler += [t_wo]
                for rr in WO_SCHED.get(r, []):
                    filler += wo_thunks(*ROUNDS[rr])
                if r == 0:
                    qn, nn = ROUNDS[1]
                    filler += load_x_thunk(xkt, qn, nn, holder, ("xk", 1))
                    filler += load_x_thunk(xvt, qn, nn, holder, ("xv", 1))
                    filler += load_x_thunk(xqt, qn, nn, holder, ("xq", 1))
                if r + 2 < NR:
                    qn2, nn2 = ROUNDS[r + 2]
                    filler += load_x_thunk(xkt, qn2, nn2, holder, ("xk", r + 2))
                    filler += load_x_thunk(xvt, qn2, nn2, holder, ("xv", r + 2))
                    filler += load_x_thunk(xqt, qn2, nn2, holder, ("xq", r + 2))
                if r + 1 < NR:
                    qn, nn = ROUNDS[r + 1]
                    filler += proj_qk_thunks(
                        qn, nn, holder, ("xq", r + 1), wq_sb, bq_sb, qhT
                    )
                    filler += proj_qk_thunks(
                        qn, nn, holder, ("xk", r + 1), wk_sb, bk_sb, khT
                    )
                    kv_deferred[r + 1] = proj_v_thunks(qn, nn, holder, ("xv", r + 1))

                ats0, ats1 = {}, {}
                jl0 = attn_jloop_thunks(qa, nq, 0, ats0)
                nm0 = attn_norm_thunks(qa, nq, 0, ats0)
                jl1 = attn_jloop_thunks(qa, nq, 1, ats1)
                cbase = qa // P
                if not last:
                    nm1 = attn_norm_thunks(qa, nq, 1, ats1)
                    att = jl0 + nm0 + jl1 + nm1
                    seg1, seg2 = att[:cbase], att[cbase:]
                    emit_interleaved(seg1, kv_filler)
                    cut = (2 * len(filler)) // 3
                    emit_interleaved(seg2[:-8], filler[:cut])
                    emit_interleaved(seg2[-8:], filler[cut:])
                else:
                    rb_holder = {}
                    act_part, mul_part = attn_norm_fast_thunks(
                        qa, nq, 1, ats1, rb_holder
                    )
                    wo_pre, wo_post = wo_tail_thunks(qa, nq)
                    att = jl0 + nm0 + jl1
                    seg1, seg2 = att[:cbase], att[cbase:]
                    emit_interleaved(seg1, kv_filler)
                    # hold one earlier-round wo unit back as PE filler for the
                    # final normalization window (it only reads old atT)
                    tail_fill, body = filler[-1:], filler[:-1]
                    emit_interleaved(seg2, body)
                    for t in act_part:
                        t()
                    for t in tail_fill:
                        t()
                    for t in wo_pre:
                        t()
                    for t in mul_part:
                        t()
                    for t in wo_post:
                        t()

    nc.compile()
    return nc


def _get_nc():
    if "nc" not in _CACHE:
        _CACHE["nc"] = _build()
    return _CACHE["nc"]


def _tile_x(x):
    # [S, E] -> x.T [(kt p), s] -> [qc, p, kt, nq] (contiguous per partition)
    a = np.ascontiguousarray(x.T).astype(np.float16)
    return np.ascontiguousarray(
        a.reshape(KT, P, S // NQMAX, NQMAX).transpose(2, 1, 0, 3)
    )


def _tile_w(w):
    # [(kt p), m] -> [p, kt, m] (contiguous per partition)
    kt = w.shape[0] // P
    a = np.ascontiguousarray(w).astype(np.float16)
    return np.ascontiguousarray(a.reshape(kt, P, w.shape[1]).transpose(1, 0, 2))


def make_in_maps(q, k, v, Wq, bq, Wk, bk, Wv, bv, Wo):
    in_maps = []
    for core in range(8):
        b, g = divmod(core, 4)
        cs = slice(g * CW, (g + 1) * CW)
        in_maps.append(
            {
                "xqt": _tile_x(q[b]),
                "xkt": _tile_x(k[b]),
                "xvt": _tile_x(v[b]),
                "wq": _tile_w(Wq[:, cs]),
                "wk": _tile_w(Wk[:, cs]),
                "wv": _tile_w(Wv[:, cs]),
                "wo": _tile_w(Wo[cs, :]),
                "bq": np.ascontiguousarray(bq[cs]),
                "bk": np.ascontiguousarray(bk[cs]),
                "bv": np.ascontiguousarray(bv[cs]).astype(np.float16),
            }
        )
    return in_maps


def run(q, k, v, Wq, bq, Wk, bk, Wv, bv, Wo, bo, **run_kwargs):
    """Returns (output, BassKernelResults)."""
    from concourse.bass_utils import run_bass_kernel_spmd

    q, k, v = (np.asarray(x, np.float32) for x in (q, k, v))
    nc = _get_nc()
    in_maps = make_in_maps(
        q, k, v,
        np.asarray(Wq, np.float32), np.asarray(bq, np.float32),
        np.asarray(Wk, np.float32), np.asarray(bk, np.float32),
        np.asarray(Wv, np.float32), np.asarray(bv, np.float32),
        np.asarray(Wo, np.float32),
    )
    res = run_bass_kernel_spmd(nc, in_maps, list(range(8)), **run_kwargs)
    out = np.zeros((B, S, E), np.float32)
    for core in range(8):
        out[core // 4] += res.results[core]["out"].astype(np.float32)
    out += np.asarray(bo, np.float32)[None, None, :]
    return out, res


def kernel(q, k, v, Wq, bq, Wk, bk, Wv, bv, Wo, bo):
    return run(q, k, v, Wq, bq, Wk, bk, Wv, bv, Wo, bo)[0]
